# revision 47
# baseline (speedup 1.0000x reference)
# Multi-head attention kernel for Trainium2, sharded over 8 NeuronCores.
#
# Sharding: core = (batch b, query-chunk qc). Each core handles QB=512 queries
# of one batch, all 12 heads, recomputing the K/V projections for its batch.
# (Cross-core dedup was measured and rejected: AllGather of the K/V quarters
# has a ~45-105us ncfw control-plane floor in this environment.)
#
# Layout strategy (bf16 matmul operands, fp32 PSUM accumulation/epilogues):
#   - Host pre-transposes activations to [E, S]; all matmul operands bf16.
#   - q^T, k^T computed as [768, S] via lhsT=W chunks; per-partition bias
#     added on DVE (keeps ScalarE free for the exp stream).
#   - Startup: few big strided DMAs (wq/xq/wk in halves, aux packed) spread
#     over the sync/gpsimd/scalar HWDGE queues; ~28 garbage warmup matmuls
#     bridge the preamble->first-real-MM gap so HAM unthrottles early.
#   - k^T projection woven with head-pair-0's scores+exp per 512-key block.
#   - v computed as [keys, 768] in two 384-wide psum chains with a ones
#     column per head ([128,16,12,65]) so PV (M=65) also yields the softmax
#     denominator row. The v loop ALSO drains hp0's PV and prefetches
#     scores+exp for hp1 AND hp2 (epool 24 tiles) so the attention pairs are
#     never ScalarE-cadence-bound (exp runs >=1 full pair ahead).
#   - Attention pairs hp1-5: per kt emit scores(hp+1) then PV(hp) consuming
#     last pair's ex tiles. PV accumulators alternate psA/psB across pairs so
#     a pair's PV starts while the previous pair's normalize still drains.
#   - normalize per pair: stage o_raw pair-stacked (partition-shifting DVE
#     copies), copy denominator rows, fast-approx DVE reciprocal, then
#     gpsimd partition_broadcast of 1/den into a [128,512] tile (head by
#     partition half) and ONE DVE multiply -> o_all. No PE involvement.
#     hp5 (tail-critical) instead uses the K=1 broadcast-matmul path into a
#     psC region + ScalarE o_raw copies to minimize latency.
#   - output projection y^T = Wo^T o per e-chunk: hp0-4 contractions emitted
#     first across all 6 chunks (2 in psA, 2 riding psC slots, 2 in psB),
#     hp5 contraction deferred so it lands right after normalize(5); bias
#     (bv@Wo + bo) added on DVE; output stored bf16 (halves writeout DMA).
#   - Notes: custom-DVE ops (reciprocal_approx_*) must NOT read PSUM; DVE ops
#     need 32-aligned base partitions; matmul accumulation groups must not
#     mix tile positions.

import numpy as np
from contextlib import ExitStack

import concourse.bass as bass
import concourse.mybir as mybir
import concourse.tile as tile
from concourse import bacc
from concourse.bass_utils import run_bass_kernel_spmd

F32 = mybir.dt.float32
BF16 = mybir.dt.bfloat16
F8 = mybir.dt.float8e4
P = 128
E = 768
S = 2048
B = 2
H = 12
D = 64
QB = 512          # queries per core
NCORES = 8
EC = E // P       # 6 e-chunks
KT = S // P       # 16 key tiles
MT_Q = E // P     # 6 M-tiles for q^T/k^T (768 rows)
NC4 = S // 512    # 4 n-slices of k^T
NAUX = 2 * MT_Q + EC  # aux cols: bq | bk | bo2


def build_nc():
    nc = bacc.Bacc("TRN2", debug=False)

    # DRAM I/O (per-core shapes; same NEFF on all 8 cores)
    # all activations/weights host-pre-arranged partition-major so every DMA
    # is per-partition contiguous (strided descriptors cap a queue ~120GB/s)
    # k-projection runs in fp8e4m3 with DoubleRow (2 fp8/PE cell, K=256 per
    # matmul): wk scaled x32 (else subnormal), kT holds 32*k, wq pre-divided
    # by 32 so scores are exact. [p, pair, j, *] interleave, j = K-half.
    xq = nc.dram_tensor("xq", (P, 3 * 2 * QB), F8, kind="ExternalInput")      # [p, pair, j, q]
    xk = nc.dram_tensor("xk", (P, NC4 * 3 * 2 * 512), F8, kind="ExternalInput")  # [p, n4, pair, j, s]
    xv = nc.dram_tensor("xv", (P, NC4 * EC * 512), BF16, kind="ExternalInput")  # [p, blk, ec, s]
    wq = nc.dram_tensor("wq", (P, 3 * 2 * E), F8, kind="ExternalInput")       # [p, pair, j, m], x32
    wk = nc.dram_tensor("wk", (P, 3 * 2 * E), F8, kind="ExternalInput")       # [p, pair, j, m], x32
    wv = nc.dram_tensor("wv", (P, EC * E), BF16, kind="ExternalInput")
    wo = nc.dram_tensor("wo", (P, EC * E), BF16, kind="ExternalInput")
    aux = nc.dram_tensor("aux", (P, NAUX), F32, kind="ExternalInput")  # bq | bk | bv@Wo+bo
    out = nc.dram_tensor("out", (E, QB), BF16, kind="ExternalOutput")  # y^T

    with tile.TileContext(nc) as tc:
        with ExitStack() as ctx:
            _emit(ctx, tc, nc, xq, xk, xv, wq, wk, wv, wo, aux, out)
    nc.compile()
    return nc


def _emit(ctx, tc, nc, xq, xk, xv, wq, wk, wv, wo, aux, out):
    # ---- pools ----
    persist = ctx.enter_context(tc.tile_pool(name="persist", bufs=1))
    wpool = ctx.enter_context(tc.tile_pool(name="wpool", bufs=2))
    xpool = ctx.enter_context(tc.tile_pool(name="xpool", bufs=2))
    xvpool = ctx.enter_context(tc.tile_pool(name="xvpool", bufs=2))
    epool = ctx.enter_context(tc.tile_pool(name="epool", bufs=32))
    dpool = ctx.enter_context(tc.tile_pool(name="dpool", bufs=1))
    outpool = ctx.enter_context(tc.tile_pool(name="outpool", bufs=4))
    # PSUM budget 8 banks/partition: psA 2 + psB 2 + psC 4
    psA = ctx.enter_context(tc.tile_pool(name="psA", bufs=2, space="PSUM"))   # [128,512]
    psB = ctx.enter_context(tc.tile_pool(name="psB", bufs=2, space="PSUM"))   # [128,512]
    psC = ctx.enter_context(tc.tile_pool(name="psC", bufs=2, space="PSUM"))   # [128,2,512]

    # ---- persistent SBUF tensors ----
    qT = persist.tile([P, MT_Q, QB], BF16)        # q^T [768, QB]
    kT = persist.tile([P, MT_Q, S], BF16)         # k^T [768, S]
    v_sb = persist.tile([P, KT, H, D + 1], BF16)  # v + ones column per head
    o_all = persist.tile([P, H // 2, QB], BF16)   # normalized o^T, pairs in partition halves
    o_raw = persist.tile([P, H // 2, QB], F32)    # unnormalized o^T, pair-stacked
    y_acc = persist.tile([P, EC, QB], F32)        # out-proj partial sums (hp0-4 woven into pairs)
    aux_sb = persist.tile([P, NAUX], F32)
    sel_sb = persist.tile([1, D], BF16)           # ones row for hp5 K=1 broadcast
    wtiny = persist.tile([P, P], BF16)            # garbage warmup operand

    def rch(dram, lo, n, w):
        # n chunks of width w starting at chunk lo, as [P, n, w]
        return dram[:, lo * w:(lo + n) * w].rearrange("p (c s) -> p c s", s=w)

    # ---- startup DMAs, ordered by first-need: HBM BW is fair-shared across
    # live queues, so only first-needed transfers may be in flight early.
    # Round 1 (needed ~8us): wq + xq halves + aux.  Round 2: wk halves +
    # xq_b.  Round 3: xk slices (woven).  wv/wo triggers are emitted later
    # and their wpool slot-waits defer them past the q/k-proj reads.
    # wq/wk arrive in m-halves ([p, mh, pair, j, 384] in dram) so the first
    # three mt-groups of each projection start ~4us earlier
    HW = 3 * 2 * (E // 2)
    wq_t = wpool.tile([P, 3, 2, E], F8, tag="w18")
    xq_t = xpool.tile([P, 3, 2, QB], F8, tag="xs")
    nc.sync.dma_start(wq_t[:, :, :, 0:384], wq[:, 0:HW].rearrange("p (c j m) -> p c j m", j=2, m=384))
    nc.gpsimd.dma_start(xq_t[:], xq[:].rearrange("p (c j q) -> p c j q", j=2, q=QB))
    nc.scalar.dma_start(aux_sb[:], aux[:])
    nc.sync.dma_start(wq_t[:, :, :, 384:768], wq[:, HW:2 * HW].rearrange("p (c j m) -> p c j m", j=2, m=384))
    wk_t = wpool.tile([P, 3, 2, E], F8, tag="w18")
    nc.scalar.dma_start(wk_t[:, :, :, 0:384], wk[:, 0:HW].rearrange("p (c j m) -> p c j m", j=2, m=384))
    nc.scalar.dma_start(wk_t[:, :, :, 384:768], wk[:, HW:2 * HW].rearrange("p (c j m) -> p c j m", j=2, m=384))

    # constants + warmup: keep PE busy across the DMA-wait so HAM unthrottles
    nc.vector.memset(wtiny[:], 0.0)
    nc.vector.memset(sel_sb[:], 1.0)
    nc.vector.memset(v_sb[:, :, :, D], 1.0)
    psW = psA.tile([P, 512], F32, tag="psA", name="warm")
    for j in range(28):
        nc.tensor.matmul(psW[:, 128 * (j % 4):128 * (j % 4) + 128], wtiny[:], wtiny[:],
                         start=True, stop=True)

    # ---- q^T projection (fp8 DoubleRow; scale 1/8192 + bias on DVE) ----
    for mt in range(MT_Q):
        ps = psA.tile([P, 512], F32, tag="psA")
        for pr in range(3):
            nc.tensor.matmul(ps[:], wq_t[:, pr, :, mt * P:(mt + 1) * P], xq_t[:, pr, :, :],
                             start=(pr == 0), stop=(pr == 2),
                             perf_mode=mybir.MatmulPerfMode.DoubleRow)
        nc.vector.tensor_scalar(qT[:, mt, :], ps[:], 1.0 / 8192.0, aux_sb[:, mt:mt + 1],
                                mybir.AluOpType.mult, mybir.AluOpType.add)

    # wv DMA: slot reuse after wq -> the trigger naturally waits until
    # q-proj's weight reads are done, keeping early HBM BW for wk/xk
    wv_t = wpool.tile([P, EC, E], BF16, tag="w18")
    nc.scalar.dma_start(wv_t[:], rch(wv, 0, 6, E))

    # ---- scores + exp helper ----
    ex_tiles = {}

    def scores_exp(hp, kt):
        st = psC.tile([P, 2, 512], F32, tag="psC")
        for i in range(2):
            po = D * i
            nc.tensor.matmul(st[:, i, :],
                             kT[po:po + D, hp, kt * P:(kt + 1) * P],
                             qT[po:po + D, hp, :],
                             start=True, stop=True)
        ex = epool.tile([P, 2, 512], BF16, tag="ex", name=f"ex{hp}_{kt}")
        nc.scalar.activation(ex[:, :, :], st[:, :, :], mybir.ActivationFunctionType.Exp)
        ex_tiles[(hp, kt)] = ex

    # ---- k^T projection woven with hp0's scores+exp ----
    for n4 in range(NC4):
        xk_t = xpool.tile([P, 3, 2, 512], F8, tag="xs")
        nc.sync.dma_start(xk_t[:], xk[:, n4 * 3072:(n4 + 1) * 3072].rearrange("p (c j s) -> p c j s", j=2, s=512))
        for mt in range(MT_Q):
            ps = psA.tile([P, 512], F32, tag="psA")
            for pr in range(3):
                nc.tensor.matmul(ps[:], wk_t[:, pr, :, mt * P:(mt + 1) * P], xk_t[:, pr, :, :],
                                 start=(pr == 0), stop=(pr == 2),
                                 perf_mode=mybir.MatmulPerfMode.DoubleRow)
            nc.vector.tensor_scalar_add(kT[:, mt, n4 * 512:(n4 + 1) * 512], ps[:],
                                        aux_sb[:, MT_Q + mt:MT_Q + mt + 1])
        for kt in range(4 * n4, 4 * n4 + 4):
            scores_exp(0, kt)

    # wo: slot reuse after wk -> trigger deferred past k-proj's weight reads
    wo_t = wpool.tile([P, EC, E], BF16, tag="w18")
    nc.sync.dma_start(wo_t[:], rch(wo, 0, EC, E))

    def pv(hp, kt, o_ps):
        ex = ex_tiles.pop((hp, kt))
        for i in range(2):
            nc.tensor.matmul(o_ps[i][0:D + 1, :],
                             v_sb[:, kt, 2 * hp + i, :],
                             ex[:, i, :],
                             start=(kt == 0), stop=(kt == KT - 1))

    def v_proj(kt, xv_t):
        for half in range(2):
            psv = psB.tile([P, 512], F32, tag="psB", name=f"psv{half}")
            for ec in range(EC):
                nc.tensor.matmul(psv[:, 0:384], xv_t[:, ec, (kt % 4) * P:(kt % 4) * P + P],
                                 wv_t[:, ec, 384 * half:384 * (half + 1)],
                                 start=(ec == 0), stop=(ec == EC - 1))
            nc.vector.tensor_copy(v_sb[:, kt, 6 * half:6 * (half + 1), 0:D],
                                  psv[:, 0:384].rearrange("p (h d) -> p h d", d=D))

    # ---- normalize ----
    def normalize(hp, o_ps):
        # dens rows -> SBUF, fast reciprocal, gpsimd partition-broadcast of
        # 1/den (both heads) to all partitions, pair-stack o_raw, DVE multiply
        # picking each partition half's head slice. partition_broadcast only
        # supports out base partition 0, so broadcast full [1,2,512].
        dens_t = dpool.tile([1, 2, 512], F32, tag="dens", name="dens")
        drec_t = dpool.tile([1, 2, 512], F32, tag="drec", name="drec")
        drec_c = dpool.tile([1, 2, 512], BF16, tag="drecc", name="drecc")
        dbc_t = dpool.tile([P, 2, 512], BF16, tag="dbc", name="dbc")
        for i in range(2):
            nc.vector.tensor_copy(dens_t[0:1, i, :], o_ps[i][D:D + 1, :])
        nc.vector.reciprocal_approx_fast(drec_t[:], dens_t[:])
        nc.vector.tensor_copy(drec_c[:], drec_t[:])
        nc.gpsimd.partition_broadcast(dbc_t[:, :, :], drec_c[0:1, :, :])
        for i in range(2):
            nc.vector.tensor_copy(o_raw[D * i:D * i + D, hp, :], o_ps[i][0:D, :])
        for i in range(2):
            nc.vector.tensor_tensor(o_all[D * i:D * i + D, hp, :],
                                    o_raw[D * i:D * i + D, hp, :],
                                    dbc_t[D * i:D * i + D, i, :],
                                    mybir.AluOpType.mult)

    def normalize_tail_pre(hp, o_ps):
        # latency-optimized variant for the last pair, part 1 (no PE work):
        # o_raw copies on ScalarE (idle by then), reciprocal + bf16 cast.
        dens_t = dpool.tile([1, 2, 512], F32, tag="dens", name="dens")
        drec_t = dpool.tile([1, 2, 512], F32, tag="drec", name="drec")
        drec_b = dpool.tile([1, 2, 512], BF16, tag="dcb", name="dcb")
        nc.vector.tensor_copy(dens_t[0:1, 0, :], o_ps[0][D:D + 1, :])
        nc.scalar.copy(dens_t[0:1, 1, :], o_ps[1][D:D + 1, :])
        nc.vector.reciprocal_approx_fast(drec_t[:], dens_t[:])
        nc.scalar.copy(drec_b[:], drec_t[:])
        for i in range(2):
            nc.scalar.copy(o_raw[D * i:D * i + D, hp, :], o_ps[i][0:D, :])
        return drec_b

    def normalize_tail_post(hp, drec_b, normtail):
        # part 2: K=1 broadcast matmuls into a psC region + DVE multiplies.
        for qh in range(2):
            for i in range(2):
                nc.tensor.matmul(normtail[D * i:D * i + D, 1, 256 * qh:256 * (qh + 1)],
                                 sel_sb[0:1, 0:D],
                                 drec_b[0:1, i, 256 * qh:256 * (qh + 1)],
                                 start=True, stop=True)
            nc.vector.tensor_tensor(o_all[:, hp, 256 * qh:256 * (qh + 1)],
                                    o_raw[:, hp, 256 * qh:256 * (qh + 1)],
                                    normtail[:, 1, 256 * qh:256 * (qh + 1)],
                                    mybir.AluOpType.mult)

    # ---- v projection + hp0 PV + scores/exp prefetch for hp1, hp2 ----
    o_ps0 = {i: psA.tile([P, 512], F32, tag="psA", name=f"o_ps{i}") for i in range(2)}
    xv_t = None
    for kt in range(KT):
        if kt % 4 == 0:
            xv_t = xvpool.tile([P, EC, 512], BF16, tag="xv")
            blk = kt // 4
            nc.gpsimd.dma_start(
                xv_t[:], xv[:, blk * EC * 512:(blk + 1) * EC * 512].rearrange("p (c s) -> p c s", s=512))
        # se3 sits after the v matmuls so its psC slot wait (exp of se1)
        # is covered by ~2us of v work instead of stalling the PE FIFO
        scores_exp(1, kt)
        scores_exp(2, kt)
        v_proj(kt, xv_t)
        scores_exp(3, kt)
        pv(0, kt, o_ps0)
    normalize(0, o_ps0)

    # out-proj partial for head pair hq, accumulated in SBUF (y_acc). Woven
    # into the NEXT pair's exp-paced window where the PE has idle cycles;
    # uses hq's own (now free) psum pool, disjoint from the running pair's.
    # out-proj partial for head pair hq, accumulated in y_acc via GpSimd
    # (idle engine; keeps the DVE queue off the pox psum-ring latency).
    # Called once per 2-kt iteration so consecutive pox ring slots have a
    # full accumulate-latency of PE work between them.
    def oproj_partial(hq, ec):
        pool, tag = (psB, "psB") if hq % 2 else (psA, "psA")
        pox = pool.tile([P, 512], F32, tag=tag, name=f"pox{ec}")
        nc.tensor.matmul(pox[:], wo_t[:, hq, ec * P:(ec + 1) * P], o_all[:, hq, :],
                         start=True, stop=True)
        if hq == 0:
            nc.vector.tensor_scalar_add(y_acc[:, ec, :], pox[:],
                                        aux_sb[:, 2 * MT_Q + ec:2 * MT_Q + ec + 1])
        else:
            nc.vector.tensor_tensor(y_acc[:, ec, :], y_acc[:, ec, :], pox[:],
                                    mybir.AluOpType.add)

    OPROJ_SCHED = {8: [0, 1], 10: [2, 3], 12: [4, 5]}

    # ---- attention pairs: consume ex produced >=1 pair earlier ----
    # kt processed two at a time with the score pairs adjacent (second
    # score-pair's fill overlaps the first's drain).
    o_ps_last = None
    for hp in range(1, H // 2):
        pool, tag = (psB, "psB") if hp % 2 else (psA, "psA")
        o_ps = {i: pool.tile([P, 512], F32, tag=tag, name=f"o_ps{i}") for i in range(2)}
        for kt2 in range(0, KT, 2):
            if hp + 3 < H // 2:
                scores_exp(hp + 3, kt2)
                scores_exp(hp + 3, kt2 + 1)
            pv(hp, kt2, o_ps)
            pv(hp, kt2 + 1, o_ps)
            for ec in OPROJ_SCHED.get(kt2, ()):
                oproj_partial(hp - 1, ec)
        if hp < H // 2 - 1:
            normalize(hp, o_ps)
        o_ps_last = o_ps

    # ---- hp5 normalize + its out-proj contraction + writeout ----
    normtail = psC.tile([P, 2, 512], F32, tag="psC", name="normtail")
    drec_b5 = normalize_tail_pre(H // 2 - 1, o_ps_last)
    normalize_tail_post(H // 2 - 1, drec_b5, normtail)
    qs_ = [nc.sync, nc.gpsimd, nc.scalar]
    for ec in range(EC):
        pool, tag = (psA, "psA") if ec % 2 == 0 else (psB, "psB")
        po = pool.tile([P, 512], F32, tag=tag, name=f"poF{ec}")
        nc.tensor.matmul(po[:], wo_t[:, H // 2 - 1, ec * P:(ec + 1) * P],
                         o_all[:, H // 2 - 1, :], start=True, stop=True)
        out_sb = outpool.tile([P, 512], BF16, tag="outsb")
        nc.vector.tensor_tensor(out_sb[:], y_acc[:, ec, :], po[:], mybir.AluOpType.add)
        qs_[ec % 3].dma_start(out[ec * P:(ec + 1) * P, :], out_sb[:])


_NC_CACHE = None


def _get_nc():
    global _NC_CACHE
    if _NC_CACHE is None:
        _NC_CACHE = build_nc()
    return _NC_CACHE


def make_in_maps(query, key_, value, Wq, bq, Wk, bk, Wv, bv, Wo, bo):
    """Host-side sharding + layout prep. Returns list of 8 input dicts."""
    query = np.asarray(query, dtype=np.float32)
    key_ = np.asarray(key_, dtype=np.float32)
    value = np.asarray(value, dtype=np.float32)
    scale = 1.0 / np.sqrt(np.float32(D))

    import ml_dtypes
    BF = ml_dtypes.bfloat16

    def pmajor_w(w):  # [E, E] -> [P, EC*E] partition-major contiguous
        return np.ascontiguousarray(
            w.reshape(EC, P, E).transpose(1, 0, 2).reshape(P, EC * E))

    def pmajor_x(xT):  # [E, S] -> [P, NC4*EC*512]: [p, slice, ec, s]
        ns = xT.shape[1] // 512
        return np.ascontiguousarray(
            xT.reshape(EC, P, ns, 512).transpose(1, 2, 0, 3).reshape(P, ns * EC * 512))

    # wq/wk stored x32 in fp8 (else subnormal); qT's DVE epilogue applies
    # 1/8192 = 1/(32 * sqrt(D) * 32), the last 32 compensating kT's x32.
    wq_f = np.transpose(np.asarray(Wq, np.float32), (1, 0, 2)).reshape(E, E)
    wk_f = np.transpose(np.asarray(Wk, np.float32), (1, 0, 2)).reshape(E, E)
    wv_f = np.transpose(np.asarray(Wv, np.float32), (1, 0, 2)).reshape(E, E)
    wo_f = np.asarray(Wo, np.float32)

    bq_f = (np.asarray(bq, np.float32).reshape(E) * (scale / 32.0)).reshape(MT_Q, P).T
    bk_f = (np.asarray(bk, np.float32).reshape(E) * 32.0).reshape(MT_Q, P).T
    bv_f = np.asarray(bv, np.float32).reshape(E)
    wo_bf32 = wo_f.astype(BF).astype(np.float32)
    bo2_f = (bv_f @ wo_bf32 + np.asarray(bo, np.float32)).reshape(EC, P).T
    aux_f = np.ascontiguousarray(np.concatenate([bq_f, bk_f, bo2_f], axis=1), dtype=np.float32)

    F8NP = ml_dtypes.float8_e4m3

    def dr_w(w):  # [E, E] -> [P, 3*2*E] DoubleRow interleave [p, mh, pair, j, m384]
        return np.ascontiguousarray(
            w.reshape(3, 2, P, 2, 384).transpose(2, 3, 0, 1, 4).reshape(P, 3 * 2 * E))

    def dr_x(xT):  # [E, S] -> [P, NC4*3*2*512]: [p, n4, pair, j, s]
        return np.ascontiguousarray(
            xT.reshape(3, 2, P, NC4, 512).transpose(2, 3, 0, 1, 4).reshape(P, NC4 * 3072))

    wq_a = dr_w(wq_f * 32.0).astype(F8NP)
    wk_a = dr_w(wk_f * 32.0).astype(F8NP)
    wv_a = pmajor_w(wv_f).astype(BF)
    wo_a = pmajor_w(wo_f).astype(BF)

    xk_a = [dr_x(key_[b].T).astype(F8NP) for b in range(B)]
    xv_a = [pmajor_x(value[b].T).astype(BF) for b in range(B)]

    in_maps = []
    for core in range(NCORES):
        b = core // (NCORES // B)
        qc = core % (NCORES // B)
        xq_T = np.ascontiguousarray(query[b, qc * QB:(qc + 1) * QB, :].T)  # [E, QB]
        xq_a = np.ascontiguousarray(
            xq_T.reshape(3, 2, P, QB).transpose(2, 0, 1, 3).reshape(P, 3 * 2 * QB)).astype(F8NP)
        in_maps.append({
            "xq": xq_a, "xk": xk_a[b], "xv": xv_a[b],
            "wq": wq_a, "wk": wk_a, "wv": wv_a, "wo": wo_a,
            "aux": aux_f,
        })
    return in_maps


def assemble(results):
    outp = np.empty((B, S, E), dtype=np.float32)
    for core in range(NCORES):
        b = core // (NCORES // B)
        qc = core % (NCORES // B)
        outp[b, qc * QB:(qc + 1) * QB, :] = results[core]["out"].T.astype(np.float32)
    return outp


def kernel(query, key_, value, Wq, bq, Wk, bk, Wv, bv, Wo, bo):
    nc = _get_nc()
    in_maps = make_in_maps(query, key_, value, Wq, bq, Wk, bk, Wv, bv, Wo, bo)
    res = run_bass_kernel_spmd(nc, in_maps, core_ids=list(range(NCORES)))
    return assemble(res.results)


# revision 49
# speedup vs baseline: 1.0144x; 1.0144x over previous
# Multi-head attention kernel for Trainium2, sharded over 8 NeuronCores.
#
# Sharding: core = (batch b, query-chunk qc). Each core handles QB=512 queries
# of one batch, all 12 heads, recomputing the K/V projections for its batch.
# (Cross-core dedup was measured and rejected: AllGather of the K/V quarters
# has a ~45-105us ncfw control-plane floor in this environment.)
#
# Layout strategy (bf16 matmul operands, fp32 PSUM accumulation/epilogues):
#   - Host pre-transposes activations to [E, S]; all matmul operands bf16.
#   - q^T, k^T computed as [768, S] via lhsT=W chunks; per-partition bias
#     added on DVE (keeps ScalarE free for the exp stream).
#   - Startup: few big strided DMAs (wq/xq/wk in halves, aux packed) spread
#     over the sync/gpsimd/scalar HWDGE queues; ~28 garbage warmup matmuls
#     bridge the preamble->first-real-MM gap so HAM unthrottles early.
#   - k^T projection woven with head-pair-0's scores+exp per 512-key block.
#   - v computed as [keys, 768] in two 384-wide psum chains with a ones
#     column per head ([128,16,12,65]) so PV (M=65) also yields the softmax
#     denominator row. The v loop ALSO drains hp0's PV and prefetches
#     scores+exp for hp1 AND hp2 (epool 24 tiles) so the attention pairs are
#     never ScalarE-cadence-bound (exp runs >=1 full pair ahead).
#   - Attention pairs hp1-5: per kt emit scores(hp+1) then PV(hp) consuming
#     last pair's ex tiles. PV accumulators alternate psA/psB across pairs so
#     a pair's PV starts while the previous pair's normalize still drains.
#   - normalize per pair: stage o_raw pair-stacked (partition-shifting DVE
#     copies), copy denominator rows, fast-approx DVE reciprocal, then
#     gpsimd partition_broadcast of 1/den into a [128,512] tile (head by
#     partition half) and ONE DVE multiply -> o_all. No PE involvement.
#     hp5 (tail-critical) instead uses the K=1 broadcast-matmul path into a
#     psC region + ScalarE o_raw copies to minimize latency.
#   - output projection y^T = Wo^T o per e-chunk: hp0-4 contractions emitted
#     first across all 6 chunks (2 in psA, 2 riding psC slots, 2 in psB),
#     hp5 contraction deferred so it lands right after normalize(5); bias
#     (bv@Wo + bo) added on DVE; output stored bf16 (halves writeout DMA).
#   - Notes: custom-DVE ops (reciprocal_approx_*) must NOT read PSUM; DVE ops
#     need 32-aligned base partitions; matmul accumulation groups must not
#     mix tile positions.

import numpy as np
from contextlib import ExitStack

import concourse.bass as bass
import concourse.mybir as mybir
import concourse.tile as tile
from concourse import bacc
from concourse.bass_utils import run_bass_kernel_spmd

F32 = mybir.dt.float32
BF16 = mybir.dt.bfloat16
F8 = mybir.dt.float8e4
P = 128
E = 768
S = 2048
B = 2
H = 12
D = 64
QB = 512          # queries per core
NCORES = 8
EC = E // P       # 6 e-chunks
KT = S // P       # 16 key tiles
MT_Q = E // P     # 6 M-tiles for q^T/k^T (768 rows)
NC4 = S // 512    # 4 n-slices of k^T
NAUX = 2 * MT_Q + EC  # aux cols: bq | bk | bo2


def build_nc():
    nc = bacc.Bacc("TRN2", debug=False)

    # DRAM I/O (per-core shapes; same NEFF on all 8 cores)
    # all activations/weights host-pre-arranged partition-major so every DMA
    # is per-partition contiguous (strided descriptors cap a queue ~120GB/s)
    # k-projection runs in fp8e4m3 with DoubleRow (2 fp8/PE cell, K=256 per
    # matmul): wk scaled x32 (else subnormal), kT holds 32*k, wq pre-divided
    # by 32 so scores are exact. [p, pair, j, *] interleave, j = K-half.
    xq = nc.dram_tensor("xq", (P, 3 * 2 * QB), F8, kind="ExternalInput")      # [p, pair, j, q]
    xk = nc.dram_tensor("xk", (P, NC4 * 3 * 2 * 512), F8, kind="ExternalInput")  # [p, n4, pair, j, s]
    xv = nc.dram_tensor("xv", (P, NC4 * EC * 512), BF16, kind="ExternalInput")  # [p, blk, ec, s]
    wq = nc.dram_tensor("wq", (P, 3 * 2 * E), F8, kind="ExternalInput")       # [p, pair, j, m], x32
    wk = nc.dram_tensor("wk", (P, 3 * 2 * E), F8, kind="ExternalInput")       # [p, pair, j, m], x32
    wv = nc.dram_tensor("wv", (P, EC * E), BF16, kind="ExternalInput")
    wo = nc.dram_tensor("wo", (P, EC * E), BF16, kind="ExternalInput")
    aux = nc.dram_tensor("aux", (P, NAUX), F32, kind="ExternalInput")  # bq | bk | bv@Wo+bo
    out = nc.dram_tensor("out", (E, QB), BF16, kind="ExternalOutput")  # y^T

    with tile.TileContext(nc) as tc:
        with ExitStack() as ctx:
            _emit(ctx, tc, nc, xq, xk, xv, wq, wk, wv, wo, aux, out)
    nc.compile()
    return nc


def _emit(ctx, tc, nc, xq, xk, xv, wq, wk, wv, wo, aux, out):
    # ---- pools ----
    persist = ctx.enter_context(tc.tile_pool(name="persist", bufs=1))
    wpool = ctx.enter_context(tc.tile_pool(name="wpool", bufs=2))
    xpool = ctx.enter_context(tc.tile_pool(name="xpool", bufs=2))
    xvpool = ctx.enter_context(tc.tile_pool(name="xvpool", bufs=2))
    epool = ctx.enter_context(tc.tile_pool(name="epool", bufs=32))
    dpool = ctx.enter_context(tc.tile_pool(name="dpool", bufs=1))
    outpool = ctx.enter_context(tc.tile_pool(name="outpool", bufs=4))
    # PSUM budget 8 banks/partition: psA 2 + psB 2 + psC 4
    psA = ctx.enter_context(tc.tile_pool(name="psA", bufs=2, space="PSUM"))   # [128,512]
    psB = ctx.enter_context(tc.tile_pool(name="psB", bufs=2, space="PSUM"))   # [128,512]
    psC = ctx.enter_context(tc.tile_pool(name="psC", bufs=2, space="PSUM"))   # [128,2,512]

    # ---- persistent SBUF tensors ----
    qT = persist.tile([P, MT_Q, QB], BF16)        # q^T [768, QB]
    kT = persist.tile([P, MT_Q, S], BF16)         # k^T [768, S]
    v_sb = persist.tile([P, KT, H, D + 1], BF16)  # v + ones column per head
    o_all = persist.tile([P, H // 2, QB], BF16)   # normalized o^T, pairs in partition halves
    o_raw = persist.tile([P, H // 2, QB], F32)    # unnormalized o^T, pair-stacked
    y_acc = persist.tile([P, EC, QB], F32)        # out-proj partial sums (hp0-4 woven into pairs)
    aux_sb = persist.tile([P, NAUX], F32)
    sel_sb = persist.tile([1, D], BF16)           # ones row for hp5 K=1 broadcast
    wtiny = persist.tile([P, P], BF16)            # garbage warmup operand

    def rch(dram, lo, n, w):
        # n chunks of width w starting at chunk lo, as [P, n, w]
        return dram[:, lo * w:(lo + n) * w].rearrange("p (c s) -> p c s", s=w)

    # ---- startup DMAs, ordered by first-need: HBM BW is fair-shared across
    # live queues, so only first-needed transfers may be in flight early.
    # Round 1 (needed ~8us): wq + xq halves + aux.  Round 2: wk halves +
    # xq_b.  Round 3: xk slices (woven).  wv/wo triggers are emitted later
    # and their wpool slot-waits defer them past the q/k-proj reads.
    wq_t = wpool.tile([P, 3, 2, E], F8, tag="w18")
    xq_t = xpool.tile([P, 3, 2, QB], F8, tag="xs")
    nc.sync.dma_start(wq_t[:], wq[:].rearrange("p (c j m) -> p c j m", j=2, m=E))
    nc.gpsimd.dma_start(xq_t[:], xq[:].rearrange("p (c j q) -> p c j q", j=2, q=QB))
    nc.scalar.dma_start(aux_sb[:], aux[:])
    wk_t = wpool.tile([P, 3, 2, E], F8, tag="w18")
    nc.sync.dma_start(wk_t[:], wk[:].rearrange("p (c j m) -> p c j m", j=2, m=E))

    # constants + warmup: keep PE busy across the DMA-wait so HAM unthrottles
    nc.vector.memset(wtiny[:], 0.0)
    nc.vector.memset(sel_sb[:], 1.0)
    nc.vector.memset(v_sb[:, :, :, D], 1.0)
    psW = psA.tile([P, 512], F32, tag="psA", name="warm")
    for j in range(28):
        nc.tensor.matmul(psW[:, 128 * (j % 4):128 * (j % 4) + 128], wtiny[:], wtiny[:],
                         start=True, stop=True)

    # ---- q^T projection (fp8 DoubleRow; scale 1/8192 + bias on DVE) ----
    for mt in range(MT_Q):
        ps = psA.tile([P, 512], F32, tag="psA")
        for pr in range(3):
            nc.tensor.matmul(ps[:], wq_t[:, pr, :, mt * P:(mt + 1) * P], xq_t[:, pr, :, :],
                             start=(pr == 0), stop=(pr == 2),
                             perf_mode=mybir.MatmulPerfMode.DoubleRow)
        nc.vector.tensor_scalar(qT[:, mt, :], ps[:], 1.0 / 8192.0, aux_sb[:, mt:mt + 1],
                                mybir.AluOpType.mult, mybir.AluOpType.add)

    # wv DMA: slot reuse after wq -> the trigger naturally waits until
    # q-proj's weight reads are done, keeping early HBM BW for wk/xk
    wv_t = wpool.tile([P, EC, E], BF16, tag="w18")
    nc.scalar.dma_start(wv_t[:], rch(wv, 0, 6, E))

    # ---- scores + exp helper ----
    ex_tiles = {}

    def scores_exp(hp, kt):
        st = psC.tile([P, 2, 512], F32, tag="psC")
        for i in range(2):
            po = D * i
            nc.tensor.matmul(st[:, i, :],
                             kT[po:po + D, hp, kt * P:(kt + 1) * P],
                             qT[po:po + D, hp, :],
                             start=True, stop=True)
        ex = epool.tile([P, 2, 512], BF16, tag="ex", name=f"ex{hp}_{kt}")
        nc.scalar.activation(ex[:, :, :], st[:, :, :], mybir.ActivationFunctionType.Exp)
        ex_tiles[(hp, kt)] = ex

    # ---- k^T projection woven with hp0's scores+exp ----
    for n4 in range(NC4):
        xk_t = xpool.tile([P, 3, 2, 512], F8, tag="xs")
        nc.sync.dma_start(xk_t[:], xk[:, n4 * 3072:(n4 + 1) * 3072].rearrange("p (c j s) -> p c j s", j=2, s=512))
        for mt in range(MT_Q):
            ps = psA.tile([P, 512], F32, tag="psA")
            for pr in range(3):
                nc.tensor.matmul(ps[:], wk_t[:, pr, :, mt * P:(mt + 1) * P], xk_t[:, pr, :, :],
                                 start=(pr == 0), stop=(pr == 2),
                                 perf_mode=mybir.MatmulPerfMode.DoubleRow)
            nc.vector.tensor_scalar_add(kT[:, mt, n4 * 512:(n4 + 1) * 512], ps[:],
                                        aux_sb[:, MT_Q + mt:MT_Q + mt + 1])
        for kt in range(4 * n4, 4 * n4 + 4):
            scores_exp(0, kt)

    # wo: slot reuse after wk -> trigger deferred past k-proj's weight reads
    wo_t = wpool.tile([P, EC, E], BF16, tag="w18")
    nc.sync.dma_start(wo_t[:], rch(wo, 0, EC, E))

    def pv(hp, kt, o_ps):
        ex = ex_tiles.pop((hp, kt))
        for i in range(2):
            nc.tensor.matmul(o_ps[i][0:D + 1, :],
                             v_sb[:, kt, 2 * hp + i, :],
                             ex[:, i, :],
                             start=(kt == 0), stop=(kt == KT - 1))

    def v_proj(kt, xv_t):
        for half in range(2):
            psv = psB.tile([P, 512], F32, tag="psB", name=f"psv{half}")
            for ec in range(EC):
                nc.tensor.matmul(psv[:, 0:384], xv_t[:, ec, (kt % 4) * P:(kt % 4) * P + P],
                                 wv_t[:, ec, 384 * half:384 * (half + 1)],
                                 start=(ec == 0), stop=(ec == EC - 1))
            nc.vector.tensor_copy(v_sb[:, kt, 6 * half:6 * (half + 1), 0:D],
                                  psv[:, 0:384].rearrange("p (h d) -> p h d", d=D))

    # ---- normalize ----
    def normalize(hp, o_ps):
        # dens rows -> SBUF, fast reciprocal, gpsimd partition-broadcast of
        # 1/den (both heads) to all partitions, pair-stack o_raw, DVE multiply
        # picking each partition half's head slice. partition_broadcast only
        # supports out base partition 0, so broadcast full [1,2,512].
        dens_t = dpool.tile([1, 2, 512], F32, tag="dens", name="dens")
        drec_t = dpool.tile([1, 2, 512], F32, tag="drec", name="drec")
        drec_c = dpool.tile([1, 2, 512], BF16, tag="drecc", name="drecc")
        dbc_t = dpool.tile([P, 2, 512], BF16, tag="dbc", name="dbc")
        for i in range(2):
            nc.vector.tensor_copy(dens_t[0:1, i, :], o_ps[i][D:D + 1, :])
        nc.vector.reciprocal_approx_fast(drec_t[:], dens_t[:])
        nc.vector.tensor_copy(drec_c[:], drec_t[:])
        nc.gpsimd.partition_broadcast(dbc_t[:, :, :], drec_c[0:1, :, :])
        for i in range(2):
            nc.vector.tensor_copy(o_raw[D * i:D * i + D, hp, :], o_ps[i][0:D, :])
        for i in range(2):
            nc.vector.tensor_tensor(o_all[D * i:D * i + D, hp, :],
                                    o_raw[D * i:D * i + D, hp, :],
                                    dbc_t[D * i:D * i + D, i, :],
                                    mybir.AluOpType.mult)

    def normalize_tail_pre(hp, o_ps):
        # latency-optimized variant for the last pair, part 1 (no PE work):
        # o_raw copies on ScalarE (idle by then), reciprocal + bf16 cast.
        dens_t = dpool.tile([1, 2, 512], F32, tag="dens", name="dens")
        drec_t = dpool.tile([1, 2, 512], F32, tag="drec", name="drec")
        drec_b = dpool.tile([1, 2, 512], BF16, tag="dcb", name="dcb")
        nc.vector.tensor_copy(dens_t[0:1, 0, :], o_ps[0][D:D + 1, :])
        nc.scalar.copy(dens_t[0:1, 1, :], o_ps[1][D:D + 1, :])
        nc.vector.reciprocal_approx_fast(drec_t[:], dens_t[:])
        nc.scalar.copy(drec_b[:], drec_t[:])
        for i in range(2):
            nc.scalar.copy(o_raw[D * i:D * i + D, hp, :], o_ps[i][0:D, :])
        return drec_b

    def normalize_tail_post(hp, drec_b, normtail):
        # part 2: K=1 broadcast matmuls into a psC region + DVE multiplies.
        for qh in range(2):
            for i in range(2):
                nc.tensor.matmul(normtail[D * i:D * i + D, 1, 256 * qh:256 * (qh + 1)],
                                 sel_sb[0:1, 0:D],
                                 drec_b[0:1, i, 256 * qh:256 * (qh + 1)],
                                 start=True, stop=True)
            nc.vector.tensor_tensor(o_all[:, hp, 256 * qh:256 * (qh + 1)],
                                    o_raw[:, hp, 256 * qh:256 * (qh + 1)],
                                    normtail[:, 1, 256 * qh:256 * (qh + 1)],
                                    mybir.AluOpType.mult)

    # ---- v projection + hp0 PV + scores/exp prefetch for hp1, hp2 ----
    o_ps0 = {i: psA.tile([P, 512], F32, tag="psA", name=f"o_ps{i}") for i in range(2)}
    xv_t = None
    for kt in range(KT):
        if kt % 4 == 0:
            xv_t = xvpool.tile([P, EC, 512], BF16, tag="xv")
            blk = kt // 4
            nc.gpsimd.dma_start(
                xv_t[:], xv[:, blk * EC * 512:(blk + 1) * EC * 512].rearrange("p (c s) -> p c s", s=512))
        # se3 sits after the v matmuls so its psC slot wait (exp of se1)
        # is covered by ~2us of v work instead of stalling the PE FIFO
        scores_exp(1, kt)
        scores_exp(2, kt)
        v_proj(kt, xv_t)
        scores_exp(3, kt)
        pv(0, kt, o_ps0)
    normalize(0, o_ps0)

    # out-proj partial for head pair hq, accumulated in SBUF (y_acc). Woven
    # into the NEXT pair's exp-paced window where the PE has idle cycles;
    # uses hq's own (now free) psum pool, disjoint from the running pair's.
    # out-proj partial for head pair hq, accumulated in y_acc via GpSimd
    # (idle engine; keeps the DVE queue off the pox psum-ring latency).
    # Called once per 2-kt iteration so consecutive pox ring slots have a
    # full accumulate-latency of PE work between them.
    def oproj_partial(hq, ec):
        pool, tag = (psB, "psB") if hq % 2 else (psA, "psA")
        pox = pool.tile([P, 512], F32, tag=tag, name=f"pox{ec}")
        nc.tensor.matmul(pox[:], wo_t[:, hq, ec * P:(ec + 1) * P], o_all[:, hq, :],
                         start=True, stop=True)
        if hq == 0:
            nc.vector.tensor_scalar_add(y_acc[:, ec, :], pox[:],
                                        aux_sb[:, 2 * MT_Q + ec:2 * MT_Q + ec + 1])
        else:
            nc.vector.tensor_tensor(y_acc[:, ec, :], y_acc[:, ec, :], pox[:],
                                    mybir.AluOpType.add)

    OPROJ_SCHED = {8: [0, 1], 10: [2, 3], 12: [4, 5]}

    # ---- attention pairs: consume ex produced >=1 pair earlier ----
    # kt processed two at a time with the score pairs adjacent (second
    # score-pair's fill overlaps the first's drain).
    o_ps_last = None
    for hp in range(1, H // 2):
        pool, tag = (psB, "psB") if hp % 2 else (psA, "psA")
        o_ps = {i: pool.tile([P, 512], F32, tag=tag, name=f"o_ps{i}") for i in range(2)}
        for kt2 in range(0, KT, 2):
            if hp + 3 < H // 2:
                scores_exp(hp + 3, kt2)
                scores_exp(hp + 3, kt2 + 1)
            pv(hp, kt2, o_ps)
            pv(hp, kt2 + 1, o_ps)
            for ec in OPROJ_SCHED.get(kt2, ()):
                oproj_partial(hp - 1, ec)
        if hp < H // 2 - 1:
            normalize(hp, o_ps)
        o_ps_last = o_ps

    # ---- hp5 normalize + its out-proj contraction + writeout ----
    normtail = psC.tile([P, 2, 512], F32, tag="psC", name="normtail")
    drec_b5 = normalize_tail_pre(H // 2 - 1, o_ps_last)
    normalize_tail_post(H // 2 - 1, drec_b5, normtail)
    qs_ = [nc.sync, nc.gpsimd, nc.scalar]
    for ec in range(EC):
        pool, tag = (psA, "psA") if ec % 2 == 0 else (psB, "psB")
        po = pool.tile([P, 512], F32, tag=tag, name=f"poF{ec}")
        nc.tensor.matmul(po[:], wo_t[:, H // 2 - 1, ec * P:(ec + 1) * P],
                         o_all[:, H // 2 - 1, :], start=True, stop=True)
        out_sb = outpool.tile([P, 512], BF16, tag="outsb")
        nc.vector.tensor_tensor(out_sb[:], y_acc[:, ec, :], po[:], mybir.AluOpType.add)
        qs_[ec % 3].dma_start(out[ec * P:(ec + 1) * P, :], out_sb[:])


_NC_CACHE = None


def _get_nc():
    global _NC_CACHE
    if _NC_CACHE is None:
        _NC_CACHE = build_nc()
    return _NC_CACHE


def make_in_maps(query, key_, value, Wq, bq, Wk, bk, Wv, bv, Wo, bo):
    """Host-side sharding + layout prep. Returns list of 8 input dicts."""
    query = np.asarray(query, dtype=np.float32)
    key_ = np.asarray(key_, dtype=np.float32)
    value = np.asarray(value, dtype=np.float32)
    scale = 1.0 / np.sqrt(np.float32(D))

    import ml_dtypes
    BF = ml_dtypes.bfloat16

    def pmajor_w(w):  # [E, E] -> [P, EC*E] partition-major contiguous
        return np.ascontiguousarray(
            w.reshape(EC, P, E).transpose(1, 0, 2).reshape(P, EC * E))

    def pmajor_x(xT):  # [E, S] -> [P, NC4*EC*512]: [p, slice, ec, s]
        ns = xT.shape[1] // 512
        return np.ascontiguousarray(
            xT.reshape(EC, P, ns, 512).transpose(1, 2, 0, 3).reshape(P, ns * EC * 512))

    # wq/wk stored x32 in fp8 (else subnormal); qT's DVE epilogue applies
    # 1/8192 = 1/(32 * sqrt(D) * 32), the last 32 compensating kT's x32.
    wq_f = np.transpose(np.asarray(Wq, np.float32), (1, 0, 2)).reshape(E, E)
    wk_f = np.transpose(np.asarray(Wk, np.float32), (1, 0, 2)).reshape(E, E)
    wv_f = np.transpose(np.asarray(Wv, np.float32), (1, 0, 2)).reshape(E, E)
    wo_f = np.asarray(Wo, np.float32)

    bq_f = (np.asarray(bq, np.float32).reshape(E) * (scale / 32.0)).reshape(MT_Q, P).T
    bk_f = (np.asarray(bk, np.float32).reshape(E) * 32.0).reshape(MT_Q, P).T
    bv_f = np.asarray(bv, np.float32).reshape(E)
    wo_bf32 = wo_f.astype(BF).astype(np.float32)
    bo2_f = (bv_f @ wo_bf32 + np.asarray(bo, np.float32)).reshape(EC, P).T
    aux_f = np.ascontiguousarray(np.concatenate([bq_f, bk_f, bo2_f], axis=1), dtype=np.float32)

    F8NP = ml_dtypes.float8_e4m3

    def dr_w(w):  # [E, E] -> [P, 3*2*E] DoubleRow interleave [p, pair, j, m]
        return np.ascontiguousarray(
            w.reshape(3, 2, P, E).transpose(2, 0, 1, 3).reshape(P, 3 * 2 * E))

    def dr_x(xT):  # [E, S] -> [P, NC4*3*2*512]: [p, n4, pair, j, s]
        return np.ascontiguousarray(
            xT.reshape(3, 2, P, NC4, 512).transpose(2, 3, 0, 1, 4).reshape(P, NC4 * 3072))

    wq_a = dr_w(wq_f * 32.0).astype(F8NP)
    wk_a = dr_w(wk_f * 32.0).astype(F8NP)
    wv_a = pmajor_w(wv_f).astype(BF)
    wo_a = pmajor_w(wo_f).astype(BF)

    xk_a = [dr_x(key_[b].T).astype(F8NP) for b in range(B)]
    xv_a = [pmajor_x(value[b].T).astype(BF) for b in range(B)]

    in_maps = []
    for core in range(NCORES):
        b = core // (NCORES // B)
        qc = core % (NCORES // B)
        xq_T = np.ascontiguousarray(query[b, qc * QB:(qc + 1) * QB, :].T)  # [E, QB]
        xq_a = np.ascontiguousarray(
            xq_T.reshape(3, 2, P, QB).transpose(2, 0, 1, 3).reshape(P, 3 * 2 * QB)).astype(F8NP)
        in_maps.append({
            "xq": xq_a, "xk": xk_a[b], "xv": xv_a[b],
            "wq": wq_a, "wk": wk_a, "wv": wv_a, "wo": wo_a,
            "aux": aux_f,
        })
    return in_maps


def assemble(results):
    outp = np.empty((B, S, E), dtype=np.float32)
    for core in range(NCORES):
        b = core // (NCORES // B)
        qc = core % (NCORES // B)
        outp[b, qc * QB:(qc + 1) * QB, :] = results[core]["out"].T.astype(np.float32)
    return outp


def kernel(query, key_, value, Wq, bq, Wk, bk, Wv, bv, Wo, bo):
    nc = _get_nc()
    in_maps = make_in_maps(query, key_, value, Wq, bq, Wk, bk, Wv, bv, Wo, bo)
    res = run_bass_kernel_spmd(nc, in_maps, core_ids=list(range(NCORES)))
    return assemble(res.results)


# revision 54
# speedup vs baseline: 1.0372x; 1.0225x over previous
# Multi-head attention kernel for Trainium2, sharded over 8 NeuronCores.
#
# Sharding: core = (batch b, query-chunk qc). Each core handles QB=512 queries
# of one batch, all 12 heads, recomputing the K/V projections for its batch.
# (Cross-core dedup was measured and rejected: AllGather of the K/V quarters
# has a ~45-105us ncfw control-plane floor in this environment.)
#
# Layout strategy (bf16 matmul operands, fp32 PSUM accumulation/epilogues):
#   - Host pre-transposes activations to [E, S]; all matmul operands bf16.
#   - q^T, k^T computed as [768, S] via lhsT=W chunks; per-partition bias
#     added on DVE (keeps ScalarE free for the exp stream).
#   - Startup: few big strided DMAs (wq/xq/wk in halves, aux packed) spread
#     over the sync/gpsimd/scalar HWDGE queues; ~28 garbage warmup matmuls
#     bridge the preamble->first-real-MM gap so HAM unthrottles early.
#   - k^T projection woven with head-pair-0's scores+exp per 512-key block.
#   - v computed as [keys, 768] in two 384-wide psum chains with a ones
#     column per head ([128,16,12,65]) so PV (M=65) also yields the softmax
#     denominator row. The v loop ALSO drains hp0's PV and prefetches
#     scores+exp for hp1 AND hp2 (epool 24 tiles) so the attention pairs are
#     never ScalarE-cadence-bound (exp runs >=1 full pair ahead).
#   - Attention pairs hp1-5: per kt emit scores(hp+1) then PV(hp) consuming
#     last pair's ex tiles. PV accumulators alternate psA/psB across pairs so
#     a pair's PV starts while the previous pair's normalize still drains.
#   - normalize per pair: stage o_raw pair-stacked (partition-shifting DVE
#     copies), copy denominator rows, fast-approx DVE reciprocal, then
#     gpsimd partition_broadcast of 1/den into a [128,512] tile (head by
#     partition half) and ONE DVE multiply -> o_all. No PE involvement.
#     hp5 (tail-critical) instead uses the K=1 broadcast-matmul path into a
#     psC region + ScalarE o_raw copies to minimize latency.
#   - output projection y^T = Wo^T o per e-chunk: hp0-4 contractions emitted
#     first across all 6 chunks (2 in psA, 2 riding psC slots, 2 in psB),
#     hp5 contraction deferred so it lands right after normalize(5); bias
#     (bv@Wo + bo) added on DVE; output stored bf16 (halves writeout DMA).
#   - Notes: custom-DVE ops (reciprocal_approx_*) must NOT read PSUM; DVE ops
#     need 32-aligned base partitions; matmul accumulation groups must not
#     mix tile positions.

import numpy as np
from contextlib import ExitStack

import concourse.bass as bass
import concourse.mybir as mybir
import concourse.tile as tile
from concourse import bacc
from concourse.bass_utils import run_bass_kernel_spmd

F32 = mybir.dt.float32
BF16 = mybir.dt.bfloat16
F8 = mybir.dt.float8e4
P = 128
E = 768
S = 2048
B = 2
H = 12
D = 64
QB = 512          # queries per core
NCORES = 8
EC = E // P       # 6 e-chunks
KT = S // P       # 16 key tiles
MT_Q = E // P     # 6 M-tiles for q^T/k^T (768 rows)
NC4 = S // 512    # 4 n-slices of k^T
NAUX = 2 * MT_Q + EC  # aux cols: bq | bk | bo2


def build_nc():
    nc = bacc.Bacc("TRN2", debug=False)

    # DRAM I/O (per-core shapes; same NEFF on all 8 cores)
    # all activations/weights host-pre-arranged partition-major so every DMA
    # is per-partition contiguous (strided descriptors cap a queue ~120GB/s)
    # k-projection runs in fp8e4m3 with DoubleRow (2 fp8/PE cell, K=256 per
    # matmul): wk scaled x32 (else subnormal), kT holds 32*k, wq pre-divided
    # by 32 so scores are exact. [p, pair, j, *] interleave, j = K-half.
    xq = nc.dram_tensor("xq", (P, 3 * 2 * QB), F8, kind="ExternalInput")      # [p, pair, j, q]
    xk = nc.dram_tensor("xk", (P, NC4 * 3 * 2 * 512), F8, kind="ExternalInput")  # [p, n4, pair, j, s]
    xv = nc.dram_tensor("xv", (P, NC4 * EC * 512), BF16, kind="ExternalInput")  # [p, blk, ec, s]
    wq = nc.dram_tensor("wq", (P, 3 * 2 * E), F8, kind="ExternalInput")       # [p, pair, j, m], x32
    wk = nc.dram_tensor("wk", (P, 3 * 2 * E), F8, kind="ExternalInput")       # [p, pair, j, m], x32
    wv = nc.dram_tensor("wv", (P, EC * E), BF16, kind="ExternalInput")
    wo = nc.dram_tensor("wo", (P, EC * E), BF16, kind="ExternalInput")
    aux = nc.dram_tensor("aux", (P, NAUX), F32, kind="ExternalInput")  # bq | bk | bv@Wo+bo
    out = nc.dram_tensor("out", (E, QB), BF16, kind="ExternalOutput")  # y^T

    with tile.TileContext(nc) as tc:
        with ExitStack() as ctx:
            _emit(ctx, tc, nc, xq, xk, xv, wq, wk, wv, wo, aux, out)
    nc.compile()
    return nc


def _emit(ctx, tc, nc, xq, xk, xv, wq, wk, wv, wo, aux, out):
    # ---- pools ----
    persist = ctx.enter_context(tc.tile_pool(name="persist", bufs=1))
    wpool = ctx.enter_context(tc.tile_pool(name="wpool", bufs=2))
    xpool = ctx.enter_context(tc.tile_pool(name="xpool", bufs=2))
    xvpool = ctx.enter_context(tc.tile_pool(name="xvpool", bufs=2))
    epool = ctx.enter_context(tc.tile_pool(name="epool", bufs=32))
    dpool = ctx.enter_context(tc.tile_pool(name="dpool", bufs=1))
    outpool = ctx.enter_context(tc.tile_pool(name="outpool", bufs=4))
    # PSUM budget 8 banks/partition: psA 2 + psB 2 + psC 4
    psA = ctx.enter_context(tc.tile_pool(name="psA", bufs=2, space="PSUM"))   # [128,512]
    psB = ctx.enter_context(tc.tile_pool(name="psB", bufs=2, space="PSUM"))   # [128,512]
    psC = ctx.enter_context(tc.tile_pool(name="psC", bufs=2, space="PSUM"))   # [128,2,512]

    # ---- persistent SBUF tensors ----
    qT = persist.tile([P, MT_Q, QB], BF16)        # q^T [768, QB]
    kT = persist.tile([P, MT_Q, S], BF16)         # k^T [768, S]
    v_sb = persist.tile([P, KT, H, D + 1], BF16)  # v + ones column per head
    o_all = persist.tile([P, H // 2, QB], BF16)   # normalized o^T, pairs in partition halves
    o_raw = persist.tile([P, H // 2, QB], F32)    # unnormalized o^T, pair-stacked
    y_acc = persist.tile([P, EC, QB], F32)        # out-proj partial sums (hp0-4 woven into pairs)
    aux_sb = persist.tile([P, NAUX], F32)
    sel_sb = persist.tile([1, D], BF16)           # ones row for hp5 K=1 broadcast
    wtiny = persist.tile([P, P], BF16)            # garbage warmup operand

    def rch(dram, lo, n, w):
        # n chunks of width w starting at chunk lo, as [P, n, w]
        return dram[:, lo * w:(lo + n) * w].rearrange("p (c s) -> p c s", s=w)

    # ---- startup DMAs, ordered by first-need: HBM BW is fair-shared across
    # live queues, so only first-needed transfers may be in flight early.
    # Round 1 (needed ~8us): wq + xq halves + aux.  Round 2: wk halves +
    # xq_b.  Round 3: xk slices (woven).  wv/wo triggers are emitted later
    # and their wpool slot-waits defer them past the q/k-proj reads.
    wq_t = wpool.tile([P, 3, 2, E], F8, tag="w18")
    xq_t = xpool.tile([P, 3, 2, QB], F8, tag="xs")
    nc.sync.dma_start(wq_t[:], wq[:].rearrange("p (c j m) -> p c j m", j=2, m=E))
    nc.gpsimd.dma_start(xq_t[:], xq[:].rearrange("p (c j q) -> p c j q", j=2, q=QB))
    nc.scalar.dma_start(aux_sb[:], aux[:])
    wk_t = wpool.tile([P, 3, 2, E], F8, tag="w18")
    nc.sync.dma_start(wk_t[:], wk[:].rearrange("p (c j m) -> p c j m", j=2, m=E))

    # constants + warmup: keep PE busy across the DMA-wait so HAM unthrottles
    nc.vector.memset(wtiny[:], 0.0)
    nc.vector.memset(sel_sb[:], 1.0)
    nc.vector.memset(v_sb[:, :, :, D], 1.0)
    psW = psA.tile([P, 512], F32, tag="psA", name="warm")
    for j in range(28):
        nc.tensor.matmul(psW[:, 128 * (j % 4):128 * (j % 4) + 128], wtiny[:], wtiny[:],
                         start=True, stop=True)

    # ---- q^T projection (fp8 DoubleRow; scale 1/8192 + bias on DVE) ----
    for mt in range(MT_Q):
        ps = psA.tile([P, 512], F32, tag="psA")
        for pr in range(3):
            nc.tensor.matmul(ps[:], wq_t[:, pr, :, mt * P:(mt + 1) * P], xq_t[:, pr, :, :],
                             start=(pr == 0), stop=(pr == 2),
                             perf_mode=mybir.MatmulPerfMode.DoubleRow)
        nc.vector.tensor_scalar(qT[:, mt, :], ps[:], 1.0 / 8192.0, aux_sb[:, mt:mt + 1],
                                mybir.AluOpType.mult, mybir.AluOpType.add)

    # wv DMA: slot reuse after wq -> the trigger naturally waits until
    # q-proj's weight reads are done, keeping early HBM BW for wk/xk
    wv_t = wpool.tile([P, EC, E], BF16, tag="w18")
    nc.scalar.dma_start(wv_t[:], rch(wv, 0, 6, E))

    # ---- scores + exp helper ----
    ex_tiles = {}

    def scores_exp(hp, kt):
        st = psC.tile([P, 2, 512], F32, tag="psC")
        for i in range(2):
            po = D * i
            nc.tensor.matmul(st[:, i, :],
                             kT[po:po + D, hp, kt * P:(kt + 1) * P],
                             qT[po:po + D, hp, :],
                             start=True, stop=True)
        ex = epool.tile([P, 2, 512], BF16, tag="ex", name=f"ex{hp}_{kt}")
        nc.scalar.activation(ex[:, :, :], st[:, :, :], mybir.ActivationFunctionType.Exp)
        ex_tiles[(hp, kt)] = ex

    # ---- k^T projection woven with hp0's scores+exp ----
    for n4 in range(NC4):
        xk_t = xpool.tile([P, 3, 2, 512], F8, tag="xs")
        nc.sync.dma_start(xk_t[:], xk[:, n4 * 3072:(n4 + 1) * 3072].rearrange("p (c j s) -> p c j s", j=2, s=512))
        for mt in range(MT_Q):
            ps = psA.tile([P, 512], F32, tag="psA")
            for pr in range(3):
                nc.tensor.matmul(ps[:], wk_t[:, pr, :, mt * P:(mt + 1) * P], xk_t[:, pr, :, :],
                                 start=(pr == 0), stop=(pr == 2),
                                 perf_mode=mybir.MatmulPerfMode.DoubleRow)
            nc.vector.tensor_scalar_add(kT[:, mt, n4 * 512:(n4 + 1) * 512], ps[:],
                                        aux_sb[:, MT_Q + mt:MT_Q + mt + 1])
        for kt in range(4 * n4, 4 * n4 + 4):
            scores_exp(0, kt)

    # wo: slot reuse after wk -> trigger deferred past k-proj's weight reads
    wo_t = wpool.tile([P, EC, E], BF16, tag="w18")
    nc.sync.dma_start(wo_t[:], rch(wo, 0, EC, E))

    def pv(hp, kt, o_ps):
        ex = ex_tiles.pop((hp, kt))
        for i in range(2):
            nc.tensor.matmul(o_ps[i][0:D + 1, :],
                             v_sb[:, kt, 2 * hp + i, :],
                             ex[:, i, :],
                             start=(kt == 0), stop=(kt == KT - 1))

    def v_proj(kt, xv_t):
        for half in range(2):
            psv = psB.tile([P, 512], F32, tag="psB", name=f"psv{half}")
            for ec in range(EC):
                nc.tensor.matmul(psv[:, 0:384], xv_t[:, ec, (kt % 4) * P:(kt % 4) * P + P],
                                 wv_t[:, ec, 384 * half:384 * (half + 1)],
                                 start=(ec == 0), stop=(ec == EC - 1))
            nc.vector.tensor_copy(v_sb[:, kt, 6 * half:6 * (half + 1), 0:D],
                                  psv[:, 0:384].rearrange("p (h d) -> p h d", d=D))

    # ---- normalize ----
    def normalize(hp, o_ps):
        # dens rows -> SBUF, fast reciprocal, gpsimd partition-broadcast of
        # 1/den (both heads) to all partitions, pair-stack o_raw, DVE multiply
        # picking each partition half's head slice. partition_broadcast only
        # supports out base partition 0, so broadcast full [1,2,512].
        dens_t = dpool.tile([1, 2, 512], F32, tag="dens", name="dens")
        drec_t = dpool.tile([1, 2, 512], F32, tag="drec", name="drec")
        dbc_t = dpool.tile([P, 2, 512], F32, tag="dbc", name="dbc")
        for i in range(2):
            nc.vector.tensor_copy(dens_t[0:1, i, :], o_ps[i][D:D + 1, :])
        nc.vector.reciprocal_approx_fast(drec_t[:], dens_t[:])
        nc.gpsimd.partition_broadcast(dbc_t[:, :, :], drec_t[0:1, :, :])
        for i in range(2):
            nc.vector.tensor_copy(o_raw[D * i:D * i + D, hp, :], o_ps[i][0:D, :])
        for i in range(2):
            nc.vector.tensor_tensor(o_all[D * i:D * i + D, hp, :],
                                    o_raw[D * i:D * i + D, hp, :],
                                    dbc_t[D * i:D * i + D, i, :],
                                    mybir.AluOpType.mult)

    def normalize_tail_pre(hp, o_ps):
        # latency-optimized variant for the last pair, part 1 (no PE work):
        # o_raw copies on ScalarE (idle by then), reciprocal + bf16 cast.
        dens_t = dpool.tile([1, 2, 512], F32, tag="dens", name="dens")
        drec_t = dpool.tile([1, 2, 512], F32, tag="drec", name="drec")
        drec_b = dpool.tile([1, 2, 512], BF16, tag="dcb", name="dcb")
        for i in range(2):
            nc.vector.tensor_copy(dens_t[0:1, i, :], o_ps[i][D:D + 1, :])
        nc.vector.reciprocal_approx_fast(drec_t[:], dens_t[:])
        nc.vector.tensor_copy(drec_b[:], drec_t[:])
        for i in range(2):
            nc.scalar.copy(o_raw[D * i:D * i + D, hp, :], o_ps[i][0:D, :])
        return drec_b

    def normalize_tail_post(hp, drec_b, normtail):
        # part 2: K=1 broadcast matmuls into a psC region + DVE multiplies.
        for qh in range(2):
            for i in range(2):
                nc.tensor.matmul(normtail[D * i:D * i + D, 1, 256 * qh:256 * (qh + 1)],
                                 sel_sb[0:1, 0:D],
                                 drec_b[0:1, i, 256 * qh:256 * (qh + 1)],
                                 start=True, stop=True)
            nc.vector.tensor_tensor(o_all[:, hp, 256 * qh:256 * (qh + 1)],
                                    o_raw[:, hp, 256 * qh:256 * (qh + 1)],
                                    normtail[:, 1, 256 * qh:256 * (qh + 1)],
                                    mybir.AluOpType.mult)

    # ---- v projection + hp0 PV + scores/exp prefetch for hp1, hp2 ----
    o_ps0 = {i: psA.tile([P, 512], F32, tag="psA", name=f"o_ps{i}") for i in range(2)}
    xv_t = None
    for kt in range(KT):
        if kt % 4 == 0:
            xv_t = xvpool.tile([P, EC, 512], BF16, tag="xv")
            blk = kt // 4
            nc.gpsimd.dma_start(
                xv_t[:], xv[:, blk * EC * 512:(blk + 1) * EC * 512].rearrange("p (c s) -> p c s", s=512))
        v_proj(kt, xv_t)
        if kt < KT - 1:
            pv(0, kt, o_ps0)
            scores_exp(1, kt)
            scores_exp(2, kt)
        else:
            scores_exp(1, kt)
            scores_exp(2, kt)
            pv(0, kt, o_ps0)
    normalize(0, o_ps0)

    # out-proj partial for head pair hq, accumulated in SBUF (y_acc). Woven
    # into the NEXT pair's exp-paced window where the PE has idle cycles;
    # uses hq's own (now free) psum pool, disjoint from the running pair's.
    # out-proj partial for head pair hq, accumulated in y_acc via GpSimd
    # (idle engine; keeps the DVE queue off the pox psum-ring latency).
    # Called once per 2-kt iteration so consecutive pox ring slots have a
    # full accumulate-latency of PE work between them.
    def oproj_partial(hq, ec):
        pool, tag = (psB, "psB") if hq % 2 else (psA, "psA")
        pox = pool.tile([P, 512], F32, tag=tag, name=f"pox{ec}")
        nc.tensor.matmul(pox[:], wo_t[:, hq, ec * P:(ec + 1) * P], o_all[:, hq, :],
                         start=True, stop=True)
        if hq == 0:
            nc.vector.tensor_scalar_add(y_acc[:, ec, :], pox[:],
                                        aux_sb[:, 2 * MT_Q + ec:2 * MT_Q + ec + 1])
        else:
            nc.vector.tensor_tensor(y_acc[:, ec, :], y_acc[:, ec, :], pox[:],
                                    mybir.AluOpType.add)

    # ---- attention pairs: consume ex produced one pair earlier ----
    o_ps_last = None
    for hp in range(1, H // 2):
        pool, tag = (psB, "psB") if hp % 2 else (psA, "psA")
        o_ps = {i: pool.tile([P, 512], F32, tag=tag, name=f"o_ps{i}") for i in range(2)}
        for kt in range(KT):
            if hp < H // 2 - 1:
                scores_exp(hp + 1, kt)
            pv(hp, kt, o_ps)
            if kt == 8:
                for ec in range(EC):
                    oproj_partial(hp - 1, ec)
        if hp < H // 2 - 1:
            normalize(hp, o_ps)
        o_ps_last = o_ps

    # ---- hp5 normalize + its out-proj contraction + writeout ----
    normtail = psC.tile([P, 2, 512], F32, tag="psC", name="normtail")
    drec_b5 = normalize_tail_pre(H // 2 - 1, o_ps_last)
    normalize_tail_post(H // 2 - 1, drec_b5, normtail)
    qs_ = [nc.sync, nc.gpsimd, nc.scalar]
    for ec in range(EC):
        po = psA.tile([P, 512], F32, tag="psA", name=f"poF{ec}")
        nc.tensor.matmul(po[:], wo_t[:, H // 2 - 1, ec * P:(ec + 1) * P],
                         o_all[:, H // 2 - 1, :], start=True, stop=True)
        out_sb = outpool.tile([P, 512], BF16, tag="outsb")
        nc.vector.tensor_tensor(out_sb[:], y_acc[:, ec, :], po[:], mybir.AluOpType.add)
        qs_[ec % 3].dma_start(out[ec * P:(ec + 1) * P, :], out_sb[:])


_NC_CACHE = None


def _get_nc():
    global _NC_CACHE
    if _NC_CACHE is None:
        _NC_CACHE = build_nc()
    return _NC_CACHE


def make_in_maps(query, key_, value, Wq, bq, Wk, bk, Wv, bv, Wo, bo):
    """Host-side sharding + layout prep. Returns list of 8 input dicts."""
    query = np.asarray(query, dtype=np.float32)
    key_ = np.asarray(key_, dtype=np.float32)
    value = np.asarray(value, dtype=np.float32)
    scale = 1.0 / np.sqrt(np.float32(D))

    import ml_dtypes
    BF = ml_dtypes.bfloat16

    def pmajor_w(w):  # [E, E] -> [P, EC*E] partition-major contiguous
        return np.ascontiguousarray(
            w.reshape(EC, P, E).transpose(1, 0, 2).reshape(P, EC * E))

    def pmajor_x(xT):  # [E, S] -> [P, NC4*EC*512]: [p, slice, ec, s]
        ns = xT.shape[1] // 512
        return np.ascontiguousarray(
            xT.reshape(EC, P, ns, 512).transpose(1, 2, 0, 3).reshape(P, ns * EC * 512))

    # wq/wk stored x32 in fp8 (else subnormal); qT's DVE epilogue applies
    # 1/8192 = 1/(32 * sqrt(D) * 32), the last 32 compensating kT's x32.
    wq_f = np.transpose(np.asarray(Wq, np.float32), (1, 0, 2)).reshape(E, E)
    wk_f = np.transpose(np.asarray(Wk, np.float32), (1, 0, 2)).reshape(E, E)
    wv_f = np.transpose(np.asarray(Wv, np.float32), (1, 0, 2)).reshape(E, E)
    wo_f = np.asarray(Wo, np.float32)

    bq_f = (np.asarray(bq, np.float32).reshape(E) * (scale / 32.0)).reshape(MT_Q, P).T
    bk_f = (np.asarray(bk, np.float32).reshape(E) * 32.0).reshape(MT_Q, P).T
    bv_f = np.asarray(bv, np.float32).reshape(E)
    wo_bf32 = wo_f.astype(BF).astype(np.float32)
    bo2_f = (bv_f @ wo_bf32 + np.asarray(bo, np.float32)).reshape(EC, P).T
    aux_f = np.ascontiguousarray(np.concatenate([bq_f, bk_f, bo2_f], axis=1), dtype=np.float32)

    F8NP = ml_dtypes.float8_e4m3

    def dr_w(w):  # [E, E] -> [P, 3*2*E] DoubleRow interleave [p, pair, j, m]
        return np.ascontiguousarray(
            w.reshape(3, 2, P, E).transpose(2, 0, 1, 3).reshape(P, 3 * 2 * E))

    def dr_x(xT):  # [E, S] -> [P, NC4*3*2*512]: [p, n4, pair, j, s]
        return np.ascontiguousarray(
            xT.reshape(3, 2, P, NC4, 512).transpose(2, 3, 0, 1, 4).reshape(P, NC4 * 3072))

    wq_a = dr_w(wq_f * 32.0).astype(F8NP)
    wk_a = dr_w(wk_f * 32.0).astype(F8NP)
    wv_a = pmajor_w(wv_f).astype(BF)
    wo_a = pmajor_w(wo_f).astype(BF)

    xk_a = [dr_x(key_[b].T).astype(F8NP) for b in range(B)]
    xv_a = [pmajor_x(value[b].T).astype(BF) for b in range(B)]

    in_maps = []
    for core in range(NCORES):
        b = core // (NCORES // B)
        qc = core % (NCORES // B)
        xq_T = np.ascontiguousarray(query[b, qc * QB:(qc + 1) * QB, :].T)  # [E, QB]
        xq_a = np.ascontiguousarray(
            xq_T.reshape(3, 2, P, QB).transpose(2, 0, 1, 3).reshape(P, 3 * 2 * QB)).astype(F8NP)
        in_maps.append({
            "xq": xq_a, "xk": xk_a[b], "xv": xv_a[b],
            "wq": wq_a, "wk": wk_a, "wv": wv_a, "wo": wo_a,
            "aux": aux_f,
        })
    return in_maps


def assemble(results):
    outp = np.empty((B, S, E), dtype=np.float32)
    for core in range(NCORES):
        b = core // (NCORES // B)
        qc = core % (NCORES // B)
        outp[b, qc * QB:(qc + 1) * QB, :] = results[core]["out"].T.astype(np.float32)
    return outp


def kernel(query, key_, value, Wq, bq, Wk, bk, Wv, bv, Wo, bo):
    nc = _get_nc()
    in_maps = make_in_maps(query, key_, value, Wq, bq, Wk, bk, Wv, bv, Wo, bo)
    res = run_bass_kernel_spmd(nc, in_maps, core_ids=list(range(NCORES)))
    return assemble(res.results)


# revision 55
# speedup vs baseline: 1.0497x; 1.0121x over previous
# Multi-head attention kernel for Trainium2, sharded over 8 NeuronCores.
#
# Sharding: core = (batch b, query-chunk qc). Each core handles QB=512 queries
# of one batch, all 12 heads, recomputing the K/V projections for its batch.
# (Cross-core dedup was measured and rejected: AllGather of the K/V quarters
# has a ~45-105us ncfw control-plane floor in this environment.)
#
# Layout/schedule (fp32 PSUM accumulation everywhere; measured ~179us/core,
# rel err 7.5e-3 vs the 2e-2 gate):
#   - q/k projections run in fp8e4m3 with DoubleRow perf mode (2 fp8 per PE
#     cell, K=256 per matmul, ~1.4x over bf16): weights stored x32 (else
#     subnormal), [p, pair, j, m] interleave with j the K-half. kT holds
#     32*k; qT's DVE epilogue applies 1/8192 = 1/(32*sqrt(D)*32) so scores
#     are exact. fp8 on v/PV/out-proj was rejected: their quantization error
#     passes straight to the output (k/q error is attenuated by softmax;
#     measured +4e-3 total for both).
#   - All dram tensors host-pre-arranged partition-major so every DMA is
#     per-partition contiguous (strided descriptors cap a queue ~120GB/s).
#     HBM (~358GB/s) is fair-shared across live queues, so transfers are
#     ordered by first-need: wq/xq/aux/wk eagerly on the three HWDGE queues,
#     xk slices woven, wv/wo triggers deferred past the q/k-proj reads via
#     the wpool bufs=2 slot ring. ~28 garbage warmup matmuls bridge the
#     preamble->first-real-MM window so HAM unthrottles early.
#   - k^T projection woven with head-pair-0's scores+exp per 512-key block
#     (ScalarE exp stream starts ~20us in; it totals ~96us and must never
#     starve: epool holds 32 ex tiles; ring order == consumption order).
#   - v computed as [keys, 768] in two 384-wide psum chains with a ones
#     column per head ([128,16,12,65]) so PV (M=65) also yields the softmax
#     denominator row (wall-optimal in bf16: a concurrent col-split pair
#     plus separate denominator matmuls costs identical PE time). The v loop
#     also drains hp0's PV and prefetches scores+exp for hp1 AND hp2.
#   - Attention pairs hp1-5: per kt emit scores(hp+1) (K=64 row-packed pairs,
#     start delta ~3ns) then PV(hp) consuming the previous pair's ex tiles.
#     PV accumulators alternate psA/psB across pairs so a pair's PV starts
#     while the previous pair's normalize still drains. The out-projection
#     contraction of pair hp-1 (6 single matmuls + DVE adds into an SBUF
#     accumulator y_acc, bias folded in at hp0) is woven into pair hp's
#     window, using hp-1's own now-free psum pool.
#   - normalize per pair: denominator rows -> SBUF, fast-approx DVE
#     reciprocal, gpsimd partition_broadcast (out base partition must be 0;
#     ~1.8us but off the critical path), o_raw pair-stacked via
#     partition-shifting DVE copies, per-half DVE multiplies -> o_all.
#     hp5 (tail-critical) uses the lower-latency K=1 broadcast-matmul path
#     into a psC region + ScalarE o_raw copies instead.
#   - tail: normalize_tail(5) -> 6 deferred hp5 matmuls (psA) -> DVE add of
#     y_acc + psum -> bf16 writeout DMA round-robin over the three queues.
#   - Notes: custom-DVE ops (reciprocal_approx_*) must NOT read PSUM; DVE
#     ops need 32-aligned base partitions; gpsimd tensor ops cannot read
#     PSUM; matmul accumulation groups must not mix tile positions. Chip
#     P0 power state sporadically downclocks PE 2.4->2.0GHz (run-to-run
#     ~179 vs ~236us for the same NEFF).

import numpy as np
from contextlib import ExitStack

import concourse.bass as bass
import concourse.mybir as mybir
import concourse.tile as tile
from concourse import bacc
from concourse.bass_utils import run_bass_kernel_spmd

F32 = mybir.dt.float32
BF16 = mybir.dt.bfloat16
F8 = mybir.dt.float8e4
P = 128
E = 768
S = 2048
B = 2
H = 12
D = 64
QB = 512          # queries per core
NCORES = 8
EC = E // P       # 6 e-chunks
KT = S // P       # 16 key tiles
MT_Q = E // P     # 6 M-tiles for q^T/k^T (768 rows)
NC4 = S // 512    # 4 n-slices of k^T
NAUX = 2 * MT_Q + EC  # aux cols: bq | bk | bo2


def build_nc():
    nc = bacc.Bacc("TRN2", debug=False)

    # DRAM I/O (per-core shapes; same NEFF on all 8 cores)
    # all activations/weights host-pre-arranged partition-major so every DMA
    # is per-partition contiguous (strided descriptors cap a queue ~120GB/s)
    # k-projection runs in fp8e4m3 with DoubleRow (2 fp8/PE cell, K=256 per
    # matmul): wk scaled x32 (else subnormal), kT holds 32*k, wq pre-divided
    # by 32 so scores are exact. [p, pair, j, *] interleave, j = K-half.
    xq = nc.dram_tensor("xq", (P, 3 * 2 * QB), F8, kind="ExternalInput")      # [p, pair, j, q]
    xk = nc.dram_tensor("xk", (P, NC4 * 3 * 2 * 512), F8, kind="ExternalInput")  # [p, n4, pair, j, s]
    xv = nc.dram_tensor("xv", (P, NC4 * EC * 512), BF16, kind="ExternalInput")  # [p, blk, ec, s]
    wq = nc.dram_tensor("wq", (P, 3 * 2 * E), F8, kind="ExternalInput")       # [p, pair, j, m], x32
    wk = nc.dram_tensor("wk", (P, 3 * 2 * E), F8, kind="ExternalInput")       # [p, pair, j, m], x32
    wv = nc.dram_tensor("wv", (P, EC * E), BF16, kind="ExternalInput")
    wo = nc.dram_tensor("wo", (P, EC * E), BF16, kind="ExternalInput")
    aux = nc.dram_tensor("aux", (P, NAUX), F32, kind="ExternalInput")  # bq | bk | bv@Wo+bo
    out = nc.dram_tensor("out", (E, QB), BF16, kind="ExternalOutput")  # y^T

    with tile.TileContext(nc) as tc:
        with ExitStack() as ctx:
            _emit(ctx, tc, nc, xq, xk, xv, wq, wk, wv, wo, aux, out)
    nc.compile()
    return nc


def _emit(ctx, tc, nc, xq, xk, xv, wq, wk, wv, wo, aux, out):
    # ---- pools ----
    persist = ctx.enter_context(tc.tile_pool(name="persist", bufs=1))
    wpool = ctx.enter_context(tc.tile_pool(name="wpool", bufs=2))
    xpool = ctx.enter_context(tc.tile_pool(name="xpool", bufs=2))
    xvpool = ctx.enter_context(tc.tile_pool(name="xvpool", bufs=2))
    epool = ctx.enter_context(tc.tile_pool(name="epool", bufs=32))
    dpool = ctx.enter_context(tc.tile_pool(name="dpool", bufs=1))
    outpool = ctx.enter_context(tc.tile_pool(name="outpool", bufs=4))
    # PSUM budget 8 banks/partition: psA 2 + psB 2 + psC 4
    psA = ctx.enter_context(tc.tile_pool(name="psA", bufs=2, space="PSUM"))   # [128,512]
    psB = ctx.enter_context(tc.tile_pool(name="psB", bufs=2, space="PSUM"))   # [128,512]
    psC = ctx.enter_context(tc.tile_pool(name="psC", bufs=2, space="PSUM"))   # [128,2,512]

    # ---- persistent SBUF tensors ----
    qT = persist.tile([P, MT_Q, QB], BF16)        # q^T [768, QB]
    kT = persist.tile([P, MT_Q, S], BF16)         # k^T [768, S]
    v_sb = persist.tile([P, KT, H, D + 1], BF16)  # v + ones column per head
    o_all = persist.tile([P, H // 2, QB], BF16)   # normalized o^T, pairs in partition halves
    o_raw = persist.tile([P, H // 2, QB], F32)    # unnormalized o^T, pair-stacked
    y_acc = persist.tile([P, EC, QB], F32)        # out-proj partial sums (hp0-4 woven into pairs)
    aux_sb = persist.tile([P, NAUX], F32)
    sel_sb = persist.tile([1, D], BF16)           # ones row for hp5 K=1 broadcast
    wtiny = persist.tile([P, P], BF16)            # garbage warmup operand

    def rch(dram, lo, n, w):
        # n chunks of width w starting at chunk lo, as [P, n, w]
        return dram[:, lo * w:(lo + n) * w].rearrange("p (c s) -> p c s", s=w)

    # ---- startup DMAs, ordered by first-need: HBM BW is fair-shared across
    # live queues, so only first-needed transfers may be in flight early.
    # Round 1 (needed ~8us): wq + xq halves + aux.  Round 2: wk halves +
    # xq_b.  Round 3: xk slices (woven).  wv/wo triggers are emitted later
    # and their wpool slot-waits defer them past the q/k-proj reads.
    wq_t = wpool.tile([P, 3, 2, E], F8, tag="w18")
    xq_t = xpool.tile([P, 3, 2, QB], F8, tag="xs")
    nc.sync.dma_start(wq_t[:], wq[:].rearrange("p (c j m) -> p c j m", j=2, m=E))
    nc.gpsimd.dma_start(xq_t[:], xq[:].rearrange("p (c j q) -> p c j q", j=2, q=QB))
    nc.scalar.dma_start(aux_sb[:], aux[:])
    wk_t = wpool.tile([P, 3, 2, E], F8, tag="w18")
    nc.sync.dma_start(wk_t[:], wk[:].rearrange("p (c j m) -> p c j m", j=2, m=E))

    # constants + warmup: keep PE busy across the DMA-wait so HAM unthrottles
    nc.vector.memset(wtiny[:], 0.0)
    nc.vector.memset(sel_sb[:], 1.0)
    nc.vector.memset(v_sb[:, :, :, D], 1.0)
    psW = psA.tile([P, 512], F32, tag="psA", name="warm")
    for j in range(28):
        nc.tensor.matmul(psW[:, 128 * (j % 4):128 * (j % 4) + 128], wtiny[:], wtiny[:],
                         start=True, stop=True)

    # ---- q^T projection (fp8 DoubleRow; scale 1/8192 + bias on DVE) ----
    for mt in range(MT_Q):
        ps = psA.tile([P, 512], F32, tag="psA")
        for pr in range(3):
            nc.tensor.matmul(ps[:], wq_t[:, pr, :, mt * P:(mt + 1) * P], xq_t[:, pr, :, :],
                             start=(pr == 0), stop=(pr == 2),
                             perf_mode=mybir.MatmulPerfMode.DoubleRow)
        nc.vector.tensor_scalar(qT[:, mt, :], ps[:], 1.0 / 8192.0, aux_sb[:, mt:mt + 1],
                                mybir.AluOpType.mult, mybir.AluOpType.add)

    # wv DMA: slot reuse after wq -> the trigger naturally waits until
    # q-proj's weight reads are done, keeping early HBM BW for wk/xk
    wv_t = wpool.tile([P, EC, E], BF16, tag="w18")
    nc.scalar.dma_start(wv_t[:], rch(wv, 0, 6, E))

    # ---- scores + exp helper ----
    ex_tiles = {}

    def scores_exp(hp, kt):
        st = psC.tile([P, 2, 512], F32, tag="psC")
        for i in range(2):
            po = D * i
            nc.tensor.matmul(st[:, i, :],
                             kT[po:po + D, hp, kt * P:(kt + 1) * P],
                             qT[po:po + D, hp, :],
                             start=True, stop=True)
        ex = epool.tile([P, 2, 512], BF16, tag="ex", name=f"ex{hp}_{kt}")
        nc.scalar.activation(ex[:, :, :], st[:, :, :], mybir.ActivationFunctionType.Exp)
        ex_tiles[(hp, kt)] = ex

    # ---- k^T projection woven with hp0's scores+exp ----
    for n4 in range(NC4):
        xk_t = xpool.tile([P, 3, 2, 512], F8, tag="xs")
        nc.sync.dma_start(xk_t[:], xk[:, n4 * 3072:(n4 + 1) * 3072].rearrange("p (c j s) -> p c j s", j=2, s=512))
        for mt in range(MT_Q):
            ps = psA.tile([P, 512], F32, tag="psA")
            for pr in range(3):
                nc.tensor.matmul(ps[:], wk_t[:, pr, :, mt * P:(mt + 1) * P], xk_t[:, pr, :, :],
                                 start=(pr == 0), stop=(pr == 2),
                                 perf_mode=mybir.MatmulPerfMode.DoubleRow)
            nc.vector.tensor_scalar_add(kT[:, mt, n4 * 512:(n4 + 1) * 512], ps[:],
                                        aux_sb[:, MT_Q + mt:MT_Q + mt + 1])
        for kt in range(4 * n4, 4 * n4 + 4):
            scores_exp(0, kt)

    # wo: slot reuse after wk -> trigger deferred past k-proj's weight reads
    wo_t = wpool.tile([P, EC, E], BF16, tag="w18")
    nc.sync.dma_start(wo_t[:], rch(wo, 0, EC, E))

    def pv(hp, kt, o_ps):
        ex = ex_tiles.pop((hp, kt))
        for i in range(2):
            nc.tensor.matmul(o_ps[i][0:D + 1, :],
                             v_sb[:, kt, 2 * hp + i, :],
                             ex[:, i, :],
                             start=(kt == 0), stop=(kt == KT - 1))

    def v_proj(kt, xv_t):
        for half in range(2):
            psv = psB.tile([P, 512], F32, tag="psB", name=f"psv{half}")
            for ec in range(EC):
                nc.tensor.matmul(psv[:, 0:384], xv_t[:, ec, (kt % 4) * P:(kt % 4) * P + P],
                                 wv_t[:, ec, 384 * half:384 * (half + 1)],
                                 start=(ec == 0), stop=(ec == EC - 1))
            nc.vector.tensor_copy(v_sb[:, kt, 6 * half:6 * (half + 1), 0:D],
                                  psv[:, 0:384].rearrange("p (h d) -> p h d", d=D))

    # ---- normalize ----
    def normalize(hp, o_ps):
        # dens rows -> SBUF, fast reciprocal, gpsimd partition-broadcast of
        # 1/den (both heads) to all partitions, pair-stack o_raw, DVE multiply
        # picking each partition half's head slice. partition_broadcast only
        # supports out base partition 0, so broadcast full [1,2,512].
        dens_t = dpool.tile([1, 2, 512], F32, tag="dens", name="dens")
        drec_t = dpool.tile([1, 2, 512], F32, tag="drec", name="drec")
        dbc_t = dpool.tile([P, 2, 512], F32, tag="dbc", name="dbc")
        for i in range(2):
            nc.vector.tensor_copy(dens_t[0:1, i, :], o_ps[i][D:D + 1, :])
        nc.vector.reciprocal_approx_fast(drec_t[:], dens_t[:])
        nc.gpsimd.partition_broadcast(dbc_t[:, :, :], drec_t[0:1, :, :])
        for i in range(2):
            nc.vector.tensor_copy(o_raw[D * i:D * i + D, hp, :], o_ps[i][0:D, :])
        for i in range(2):
            nc.vector.tensor_tensor(o_all[D * i:D * i + D, hp, :],
                                    o_raw[D * i:D * i + D, hp, :],
                                    dbc_t[D * i:D * i + D, i, :],
                                    mybir.AluOpType.mult)

    def normalize_tail_pre(hp, o_ps):
        # latency-optimized variant for the last pair, part 1 (no PE work):
        # o_raw copies on ScalarE (idle by then), reciprocal + bf16 cast.
        dens_t = dpool.tile([1, 2, 512], F32, tag="dens", name="dens")
        drec_t = dpool.tile([1, 2, 512], F32, tag="drec", name="drec")
        drec_b = dpool.tile([1, 2, 512], BF16, tag="dcb", name="dcb")
        for i in range(2):
            nc.vector.tensor_copy(dens_t[0:1, i, :], o_ps[i][D:D + 1, :])
        nc.vector.reciprocal_approx_fast(drec_t[:], dens_t[:])
        nc.vector.tensor_copy(drec_b[:], drec_t[:])
        for i in range(2):
            nc.scalar.copy(o_raw[D * i:D * i + D, hp, :], o_ps[i][0:D, :])
        return drec_b

    def normalize_tail_post(hp, drec_b, normtail):
        # part 2: K=1 broadcast matmuls into a psC region + DVE multiplies.
        for qh in range(2):
            for i in range(2):
                nc.tensor.matmul(normtail[D * i:D * i + D, 1, 256 * qh:256 * (qh + 1)],
                                 sel_sb[0:1, 0:D],
                                 drec_b[0:1, i, 256 * qh:256 * (qh + 1)],
                                 start=True, stop=True)
            nc.vector.tensor_tensor(o_all[:, hp, 256 * qh:256 * (qh + 1)],
                                    o_raw[:, hp, 256 * qh:256 * (qh + 1)],
                                    normtail[:, 1, 256 * qh:256 * (qh + 1)],
                                    mybir.AluOpType.mult)

    # ---- v projection + hp0 PV + scores/exp prefetch for hp1, hp2 ----
    o_ps0 = {i: psA.tile([P, 512], F32, tag="psA", name=f"o_ps{i}") for i in range(2)}
    xv_t = None
    for kt in range(KT):
        if kt % 4 == 0:
            xv_t = xvpool.tile([P, EC, 512], BF16, tag="xv")
            blk = kt // 4
            nc.gpsimd.dma_start(
                xv_t[:], xv[:, blk * EC * 512:(blk + 1) * EC * 512].rearrange("p (c s) -> p c s", s=512))
        v_proj(kt, xv_t)
        if kt < KT - 1:
            pv(0, kt, o_ps0)
            scores_exp(1, kt)
            scores_exp(2, kt)
        else:
            scores_exp(1, kt)
            scores_exp(2, kt)
            pv(0, kt, o_ps0)
    normalize(0, o_ps0)

    # out-proj partial for head pair hq, accumulated in SBUF (y_acc). Woven
    # into the NEXT pair's exp-paced window where the PE has idle cycles;
    # uses hq's own (now free) psum pool, disjoint from the running pair's.
    # out-proj partial for head pair hq, accumulated in y_acc via GpSimd
    # (idle engine; keeps the DVE queue off the pox psum-ring latency).
    # Called once per 2-kt iteration so consecutive pox ring slots have a
    # full accumulate-latency of PE work between them.
    def oproj_partial(hq, ec):
        pool, tag = (psB, "psB") if hq % 2 else (psA, "psA")
        pox = pool.tile([P, 512], F32, tag=tag, name=f"pox{ec}")
        nc.tensor.matmul(pox[:], wo_t[:, hq, ec * P:(ec + 1) * P], o_all[:, hq, :],
                         start=True, stop=True)
        if hq == 0:
            nc.vector.tensor_scalar_add(y_acc[:, ec, :], pox[:],
                                        aux_sb[:, 2 * MT_Q + ec:2 * MT_Q + ec + 1])
        else:
            nc.vector.tensor_tensor(y_acc[:, ec, :], y_acc[:, ec, :], pox[:],
                                    mybir.AluOpType.add)

    # ---- attention pairs: consume ex produced one pair earlier ----
    o_ps_last = None
    for hp in range(1, H // 2):
        pool, tag = (psB, "psB") if hp % 2 else (psA, "psA")
        o_ps = {i: pool.tile([P, 512], F32, tag=tag, name=f"o_ps{i}") for i in range(2)}
        for kt in range(KT):
            if hp < H // 2 - 1:
                scores_exp(hp + 1, kt)
            pv(hp, kt, o_ps)
            if kt == 8:
                for ec in range(EC):
                    oproj_partial(hp - 1, ec)
        if hp < H // 2 - 1:
            normalize(hp, o_ps)
        o_ps_last = o_ps

    # ---- hp5 normalize + its out-proj contraction + writeout ----
    normtail = psC.tile([P, 2, 512], F32, tag="psC", name="normtail")
    drec_b5 = normalize_tail_pre(H // 2 - 1, o_ps_last)
    normalize_tail_post(H // 2 - 1, drec_b5, normtail)
    qs_ = [nc.sync, nc.gpsimd, nc.scalar]
    for ec in range(EC):
        po = psA.tile([P, 512], F32, tag="psA", name=f"poF{ec}")
        nc.tensor.matmul(po[:], wo_t[:, H // 2 - 1, ec * P:(ec + 1) * P],
                         o_all[:, H // 2 - 1, :], start=True, stop=True)
        out_sb = outpool.tile([P, 512], BF16, tag="outsb")
        nc.vector.tensor_tensor(out_sb[:], y_acc[:, ec, :], po[:], mybir.AluOpType.add)
        qs_[ec % 3].dma_start(out[ec * P:(ec + 1) * P, :], out_sb[:])


_NC_CACHE = None


def _get_nc():
    global _NC_CACHE
    if _NC_CACHE is None:
        _NC_CACHE = build_nc()
    return _NC_CACHE


def make_in_maps(query, key_, value, Wq, bq, Wk, bk, Wv, bv, Wo, bo):
    """Host-side sharding + layout prep. Returns list of 8 input dicts."""
    query = np.asarray(query, dtype=np.float32)
    key_ = np.asarray(key_, dtype=np.float32)
    value = np.asarray(value, dtype=np.float32)
    scale = 1.0 / np.sqrt(np.float32(D))

    import ml_dtypes
    BF = ml_dtypes.bfloat16

    def pmajor_w(w):  # [E, E] -> [P, EC*E] partition-major contiguous
        return np.ascontiguousarray(
            w.reshape(EC, P, E).transpose(1, 0, 2).reshape(P, EC * E))

    def pmajor_x(xT):  # [E, S] -> [P, NC4*EC*512]: [p, slice, ec, s]
        ns = xT.shape[1] // 512
        return np.ascontiguousarray(
            xT.reshape(EC, P, ns, 512).transpose(1, 2, 0, 3).reshape(P, ns * EC * 512))

    # wq/wk stored x32 in fp8 (else subnormal); qT's DVE epilogue applies
    # 1/8192 = 1/(32 * sqrt(D) * 32), the last 32 compensating kT's x32.
    wq_f = np.transpose(np.asarray(Wq, np.float32), (1, 0, 2)).reshape(E, E)
    wk_f = np.transpose(np.asarray(Wk, np.float32), (1, 0, 2)).reshape(E, E)
    wv_f = np.transpose(np.asarray(Wv, np.float32), (1, 0, 2)).reshape(E, E)
    wo_f = np.asarray(Wo, np.float32)

    bq_f = (np.asarray(bq, np.float32).reshape(E) * (scale / 32.0)).reshape(MT_Q, P).T
    bk_f = (np.asarray(bk, np.float32).reshape(E) * 32.0).reshape(MT_Q, P).T
    bv_f = np.asarray(bv, np.float32).reshape(E)
    wo_bf32 = wo_f.astype(BF).astype(np.float32)
    bo2_f = (bv_f @ wo_bf32 + np.asarray(bo, np.float32)).reshape(EC, P).T
    aux_f = np.ascontiguousarray(np.concatenate([bq_f, bk_f, bo2_f], axis=1), dtype=np.float32)

    F8NP = ml_dtypes.float8_e4m3

    def dr_w(w):  # [E, E] -> [P, 3*2*E] DoubleRow interleave [p, pair, j, m]
        return np.ascontiguousarray(
            w.reshape(3, 2, P, E).transpose(2, 0, 1, 3).reshape(P, 3 * 2 * E))

    def dr_x(xT):  # [E, S] -> [P, NC4*3*2*512]: [p, n4, pair, j, s]
        return np.ascontiguousarray(
            xT.reshape(3, 2, P, NC4, 512).transpose(2, 3, 0, 1, 4).reshape(P, NC4 * 3072))

    wq_a = dr_w(wq_f * 32.0).astype(F8NP)
    wk_a = dr_w(wk_f * 32.0).astype(F8NP)
    wv_a = pmajor_w(wv_f).astype(BF)
    wo_a = pmajor_w(wo_f).astype(BF)

    xk_a = [dr_x(key_[b].T).astype(F8NP) for b in range(B)]
    xv_a = [pmajor_x(value[b].T).astype(BF) for b in range(B)]

    in_maps = []
    for core in range(NCORES):
        b = core // (NCORES // B)
        qc = core % (NCORES // B)
        xq_T = np.ascontiguousarray(query[b, qc * QB:(qc + 1) * QB, :].T)  # [E, QB]
        xq_a = np.ascontiguousarray(
            xq_T.reshape(3, 2, P, QB).transpose(2, 0, 1, 3).reshape(P, 3 * 2 * QB)).astype(F8NP)
        in_maps.append({
            "xq": xq_a, "xk": xk_a[b], "xv": xv_a[b],
            "wq": wq_a, "wk": wk_a, "wv": wv_a, "wo": wo_a,
            "aux": aux_f,
        })
    return in_maps


def assemble(results):
    outp = np.empty((B, S, E), dtype=np.float32)
    for core in range(NCORES):
        b = core // (NCORES // B)
        qc = core % (NCORES // B)
        outp[b, qc * QB:(qc + 1) * QB, :] = results[core]["out"].T.astype(np.float32)
    return outp


def kernel(query, key_, value, Wq, bq, Wk, bk, Wv, bv, Wo, bo):
    nc = _get_nc()
    in_maps = make_in_maps(query, key_, value, Wq, bq, Wk, bk, Wv, bv, Wo, bo)
    res = run_bass_kernel_spmd(nc, in_maps, core_ids=list(range(NCORES)))
    return assemble(res.results)
